# revision 14
# baseline (speedup 1.0000x reference)
"""GRU ActorNetwork Trainium2 kernel.

B=256, S=512, I=256, H=128, A=18. Data-parallel over batch: 8 cores x 32 batch.
Per core (everything in "transposed" orientation: feature dim on partitions,
(time*batch) on the free dim):

  phase A : x_projT[g] = W_ih[g] @ obsT + bias   (bf16 matmuls, f32 psum)
  recur   : 512 sequential GRU steps, H=128 partitions, 32 batch free
            - psum_rz pre-init with [xr|xz] via identity matmul (start=True),
              gate matmuls accumulate on top -> sigmoid reads PSUM directly
            - n gate: u = (psum_n + b_hn) * r   via fused scalar_tensor_tensor
  head    : logitsT = W_head @ hT (+ b_head), per 16-step chunk
Host side only does layout transforms (transpose / cast / shard).
"""

import numpy as np
import ml_dtypes

import concourse.bass as bass
import concourse.mybir as mybir
from concourse import tile
from concourse.bass_utils import run_bass_kernel_spmd

BF16 = mybir.dt.bfloat16
F32 = mybir.dt.float32
AF = mybir.ActivationFunctionType
ALU = mybir.AluOpType

B, S, I, H, A = 256, 512, 256, 128, 18
NCORES = 8
BL = B // NCORES          # 32 batch per core
CH = 16                   # recurrence steps per chunk (16*32 = 512 free)
NCH = S // CH             # 32 chunks
FREE = CH * BL            # 512

nbf = ml_dtypes.bfloat16


def build_nc():
    nc = bass.Bass()

    obsT = nc.dram_tensor("obsT", [2, 128, S * BL], BF16, kind="ExternalInput")
    wih = nc.dram_tensor("wih", [2, 128, 3 * H], BF16, kind="ExternalInput")
    whh = nc.dram_tensor("whh", [128, 3 * H], BF16, kind="ExternalInput")
    whead = nc.dram_tensor("whead", [128, A], BF16, kind="ExternalInput")
    pa_bias = nc.dram_tensor("pa_bias", [1, 3 * H], BF16, kind="ExternalInput")
    bh_col = nc.dram_tensor("bh_col", [A, 1], F32, kind="ExternalInput")
    ones = nc.dram_tensor("ones", [1, FREE], BF16, kind="ExternalInput")
    ident = nc.dram_tensor("ident", [128, 128], BF16, kind="ExternalInput")
    bn_bc = nc.dram_tensor("bn_bc", [128, BL], BF16, kind="ExternalInput")
    h0T = nc.dram_tensor("h0T", [128, BL], BF16, kind="ExternalInput")

    logitsT = nc.dram_tensor("logitsT", [A, S * BL], F32, kind="ExternalOutput")
    hnT = nc.dram_tensor("hnT", [128, BL], F32, kind="ExternalOutput")

    with tile.TileContext(nc) as tc:
        with (
            tc.tile_pool(name="const", bufs=1) as constp,
            tc.tile_pool(name="big", bufs=1) as bigp,
            tc.tile_pool(name="obs", bufs=4) as obsp,
            tc.tile_pool(name="step", bufs=6) as stepp,
            tc.tile_pool(name="lout", bufs=3) as loutp,
            tc.tile_pool(name="pa_ps", bufs=2, space="PSUM") as pa_psp,
            tc.tile_pool(name="n_ps", bufs=1, space="PSUM") as n_psp,
            tc.tile_pool(name="hd_ps", bufs=1, space="PSUM") as hd_psp,
        ):
            # ---- persistent SBUF ----
            wih_sb = constp.tile([128, 2, 3 * H], BF16, tag="wih")
            whh_sb = constp.tile([128, 3 * H], BF16, tag="whh")
            whead_sb = constp.tile([128, A], BF16, tag="whead")
            pab_sb = constp.tile([1, 3 * H], BF16, tag="pab")
            bh_sb = constp.tile([A, 1], F32, tag="bh")
            ones_sb = constp.tile([1, FREE], BF16, tag="ones")
            ident_sb = constp.tile([128, 128], BF16, tag="ident")
            bnbc_sb = constp.tile([128, BL], BF16, tag="bnbc")

            hT = bigp.tile([128, S + 1, BL], BF16, tag="hT")

            for k in range(2):
                nc.gpsimd.dma_start(wih_sb[:, k, :], wih[k])
            nc.gpsimd.dma_start(whh_sb[:], whh[:])
            nc.gpsimd.dma_start(whead_sb[:], whead[:])
            nc.gpsimd.dma_start(pab_sb[:], pa_bias[:])
            nc.gpsimd.dma_start(bh_sb[:], bh_col[:])
            nc.gpsimd.dma_start(ones_sb[:], ones[:])
            nc.gpsimd.dma_start(ident_sb[:], ident[:])
            nc.gpsimd.dma_start(bnbc_sb[:], bn_bc[:])
            nc.gpsimd.dma_start(hT[:, 0, :], h0T[:])

            # x-projection PSUM banks per chunk: gate matmuls of the
            # recurrence accumulate directly into these (has_written was
            # set by the phase-A matmuls), so sigmoid reads PSUM and no
            # evacuation pass exists at all.
            pa_tiles = {}
            pa_obs = {}

            def phase_a_dma(c):
                obs_sb = [
                    obsp.tile([128, FREE], BF16, tag=f"obs{k}", name=f"obs{k}")
                    for k in range(2)
                ]
                for k in range(2):
                    nc.gpsimd.dma_start(
                        obs_sb[k][:], obsT[k, :, c * FREE:(c + 1) * FREE]
                    )
                tiles = []
                for gn in ["r", "z", "n"]:
                    ps = pa_psp.tile([128, FREE], F32, tag=f"pa_{gn}",
                                     name=f"pa_{gn}")
                    tiles.append(ps)
                pa_tiles[c] = tiles
                pa_obs[c] = obs_sb

            def phase_a_piece(c, i):
                g, j = divmod(i, 3)
                ps = pa_tiles[c][g]
                obs_sb = pa_obs[c]
                if j == 0:
                    nc.tensor.matmul(
                        ps[:], pab_sb[:, g * H:(g + 1) * H], ones_sb[:],
                        start=True, stop=False,
                    )
                else:
                    nc.tensor.matmul(
                        ps[:], wih_sb[:, j - 1, g * H:(g + 1) * H],
                        obs_sb[j - 1][:], start=False, stop=False,
                    )

            def step(t):
                c, o = divmod(t, CH)
                pa_r, pa_z, pa_n = pa_tiles[c]
                xr = pa_r[:, o * BL:(o + 1) * BL]
                xz = pa_z[:, o * BL:(o + 1) * BL]
                xn = pa_n[:, o * BL:(o + 1) * BL]
                n_ps = n_psp.tile([128, BL], F32, tag="n")
                r_sb = stepp.tile([128, BL], BF16, tag="r")
                z_sb = stepp.tile([128, BL], BF16, tag="z")
                zc_sb = stepp.tile([128, BL], BF16, tag="zc")
                u_sb = stepp.tile([128, BL], BF16, tag="u")
                q_sb = stepp.tile([128, BL], BF16, tag="q")
                nn_sb = stepp.tile([128, BL], BF16, tag="nn")
                zh_sb = stepp.tile([128, BL], BF16, tag="zh")
                t1_sb = stepp.tile([128, BL], BF16, tag="t1")

                h_prev = hT[:, t, :]
                nc.tensor.matmul(xr, whh_sb[:, 0:H], h_prev,
                                 start=False, stop=False,
                                 skip_group_check=True)
                nc.tensor.matmul(n_ps[:], ident_sb[:], bnbc_sb[:],
                                 start=True, stop=False)
                nc.tensor.matmul(n_ps[:], whh_sb[:, 2 * H:3 * H], h_prev,
                                 start=False, stop=True)
                nc.tensor.matmul(xz, whh_sb[:, H:2 * H], h_prev,
                                 start=False, stop=(o == CH - 1),
                                 skip_group_check=True)

                nc.scalar.activation(r_sb[:], xr, AF.Sigmoid)
                # u = (Wn h + b_hn) * r   (b_hn pre-loaded into psum)
                nc.vector.tensor_mul(u_sb[:], n_ps[:], r_sb[:])
                nc.vector.tensor_add(q_sb[:], u_sb[:], xn)
                nc.scalar.activation(nn_sb[:], q_sb[:], AF.Tanh)

                nc.scalar.activation(z_sb[:], xz, AF.Sigmoid)
                nc.vector.tensor_scalar(zc_sb[:], z_sb[:],
                                        -1.0, 1.0, op0=ALU.mult, op1=ALU.add)
                nc.vector.tensor_mul(zh_sb[:], z_sb[:], h_prev)
                nc.vector.tensor_mul(t1_sb[:], zc_sb[:], nn_sb[:])
                nc.vector.tensor_add(hT[:, t + 1, :], t1_sb[:], zh_sb[:])

            head_tiles = {}

            def head_mm(c):
                hd_ps = hd_psp.tile([A, FREE], F32, tag="hd")
                lo_sb = loutp.tile([A, FREE], F32, tag="lo")
                hv = hT[:, 1 + c * CH:1 + (c + 1) * CH, :].rearrange(
                    "p t b -> p (t b)")
                nc.tensor.matmul(hd_ps[:], whead_sb[:], hv, start=True,
                                 stop=True)
                head_tiles[c] = (hd_ps, lo_sb)

            def head_evac(c, j):
                hd_ps, lo_sb = head_tiles[c]
                nc.vector.tensor_scalar_add(
                    lo_sb[:, j * 128:(j + 1) * 128],
                    hd_ps[:, j * 128:(j + 1) * 128], bh_sb[:, 0:1])

            def head_dma(c):
                _, lo_sb = head_tiles[c]
                nc.gpsimd.dma_start(
                    logitsT[:, c * FREE:(c + 1) * FREE], lo_sb[:])

            phase_a_dma(0)
            phase_a_dma(1)
            for i in range(9):
                phase_a_piece(0, i)
            for c in range(NCH):
                if c + 2 <= NCH - 1:
                    phase_a_dma(c + 2)
                for o in range(CH):
                    step(c * CH + o)
                    if c + 1 <= NCH - 1 and o < 9:
                        phase_a_piece(c + 1, o)
                    if c >= 1:
                        if o == 10:
                            head_mm(c - 1)
                        elif 11 <= o <= 14:
                            head_evac(c - 1, o - 11)
                        elif o == 15:
                            head_dma(c - 1)
            head_mm(NCH - 1)
            for j in range(4):
                head_evac(NCH - 1, j)
            head_dma(NCH - 1)
            hn_sb = constp.tile([128, BL], F32, tag="hn")
            nc.vector.tensor_copy(hn_sb[:], hT[:, S, :])
            nc.gpsimd.dma_start(hnT[:], hn_sb[:])

    _split_multi_waits(nc)
    return nc


_WAIT_TEMPLATE = None


def _get_wait_template():
    global _WAIT_TEMPLATE
    if _WAIT_TEMPLATE is None:
        scratch = bass.Bass()
        with scratch.semaphore() as sem:
            bi = scratch.vector.wait_ge(sem, 1)
        _WAIT_TEMPLATE = bi.ins
    return _WAIT_TEMPLATE


def _split_multi_waits(nc):
    """Walrus on this toolchain rejects instructions with >1 sync wait.
    Move extra waits onto standalone same-engine EventSemaphore
    instructions directly before the instruction (engines are in-order,
    so semantics are preserved).  DMAs must be SWDGE (gpsimd-issued) for
    this to be sound; HWDGE descriptors are not gated by engine order."""
    import copy

    tmpl = _get_wait_template()
    f = nc.m.functions[0]
    n = [0]
    for blk in f.blocks:
        out = []
        for inst in blk.instructions:
            si = inst.sync_info
            waits = list(si.on_wait or []) if si else []
            if len(waits) > 1:
                if type(inst).__name__ == "InstDMACopy":
                    assert str(inst.engine) in ("EngineType.Pool",), (
                        f"multi-wait HWDGE DMA {inst.name} engine {inst.engine}"
                    )
                for w in waits[:-1]:
                    wi = copy.deepcopy(tmpl)
                    n[0] += 1
                    wi.name = f"I-wsplit-{n[0]}"
                    wi.engine = inst.engine
                    wi.sync_info = mybir.SyncInfo(on_wait=[w], on_update=[])
                    out.append(wi)
                inst.sync_info = mybir.SyncInfo(
                    on_wait=[waits[-1]], on_update=list(si.on_update or [])
                )
            out.append(inst)
        try:
            blk.instructions[:] = out
        except TypeError:
            blk.instructions = out
    return n[0]


def _prep_shared(W_ih, W_hh, b_ih, b_hh, W_head, b_head):
    wih = np.ascontiguousarray(W_ih.T.reshape(2, 128, 3 * H)).astype(nbf)
    whh = np.ascontiguousarray(W_hh.T).astype(nbf)
    whead = np.ascontiguousarray(W_head.T).astype(nbf)
    pa_bias = np.concatenate(
        [
            b_ih[0:H] + b_hh[0:H],
            b_ih[H:2 * H] + b_hh[H:2 * H],
            b_ih[2 * H:3 * H],
        ]
    ).reshape(1, 3 * H).astype(nbf)
    bh = np.ascontiguousarray(b_head.reshape(A, 1)).astype(np.float32)
    ones = np.ones((1, FREE), dtype=nbf)
    ident = np.eye(128, dtype=nbf)
    bn_bc = np.ascontiguousarray(
        np.broadcast_to(b_hh[2 * H:3 * H].reshape(H, 1), (H, BL))).astype(nbf)
    return dict(wih=wih, whh=whh, whead=whead,
                pa_bias=pa_bias, bh_col=bh, ones=ones, ident=ident,
                bn_bc=bn_bc)


last_in_maps = None
_nc_cache = []


def _nc_cache_get():
    if not _nc_cache:
        _nc_cache.append(build_nc())
    return _nc_cache[0]


def kernel(obs_seq, h0, W_ih, W_hh, b_ih, b_hh, W_head, b_head):
    global last_in_maps
    obs_seq = np.asarray(obs_seq, dtype=np.float32)
    h0 = np.asarray(h0, dtype=np.float32)
    shared = _prep_shared(
        np.asarray(W_ih, np.float32), np.asarray(W_hh, np.float32),
        np.asarray(b_ih, np.float32), np.asarray(b_hh, np.float32),
        np.asarray(W_head, np.float32), np.asarray(b_head, np.float32))

    in_maps = []
    for k in range(NCORES):
        ob = obs_seq[k * BL:(k + 1) * BL]          # (32, 512, 256)
        obT = np.ascontiguousarray(ob.transpose(2, 1, 0))  # (256, 512, 32)
        obT = obT.reshape(2, 128, S * BL).astype(nbf)
        h0T = np.ascontiguousarray(h0[0, k * BL:(k + 1) * BL, :].T).astype(nbf)
        m = dict(shared)
        m["obsT"] = obT
        m["h0T"] = h0T
        in_maps.append(m)
    last_in_maps = in_maps

    nc = _nc_cache_get()

    res = run_bass_kernel_spmd(nc, in_maps, core_ids=list(range(NCORES)))
    logits = np.empty((B, S, A), dtype=np.float32)
    hn = np.empty((1, B, H), dtype=np.float32)
    for k in range(NCORES):
        lt = res.results[k]["logitsT"].reshape(A, S, BL)
        logits[k * BL:(k + 1) * BL] = lt.transpose(2, 1, 0)
        hn[0, k * BL:(k + 1) * BL, :] = res.results[k]["hnT"].T
    return logits, hn


# revision 15
# speedup vs baseline: 1.2241x; 1.2241x over previous
"""GRU ActorNetwork Trainium2 kernel.

B=256, S=512, I=256, H=128, A=18. Data-parallel over batch: 8 cores x 32 batch.
Per core (everything in "transposed" orientation: feature dim on partitions,
(time*batch) on the free dim):

  phase A : x_projT[g] = W_ih[g] @ obsT + bias   (bf16 matmuls, f32 psum)
  recur   : 512 sequential GRU steps, H=128 partitions, 32 batch free
            - psum_rz pre-init with [xr|xz] via identity matmul (start=True),
              gate matmuls accumulate on top -> sigmoid reads PSUM directly
            - n gate: u = (psum_n + b_hn) * r   via fused scalar_tensor_tensor
  head    : logitsT = W_head @ hT (+ b_head), per 16-step chunk
Host side only does layout transforms (transpose / cast / shard).
"""

import numpy as np
import ml_dtypes

import concourse.bass as bass
import concourse.mybir as mybir
from concourse import tile
from concourse.bass_utils import run_bass_kernel_spmd

BF16 = mybir.dt.bfloat16
F32 = mybir.dt.float32
AF = mybir.ActivationFunctionType
ALU = mybir.AluOpType

B, S, I, H, A = 256, 512, 256, 128, 18
NCORES = 8
BL = B // NCORES          # 32 batch per core
CH = 16                   # recurrence steps per chunk (16*32 = 512 free)
NCH = S // CH             # 32 chunks
FREE = CH * BL            # 512

nbf = ml_dtypes.bfloat16


def build_nc():
    nc = bass.Bass()

    obsT = nc.dram_tensor("obsT", [2, 128, S * BL], BF16, kind="ExternalInput")
    wih = nc.dram_tensor("wih", [2, 128, 3 * H], BF16, kind="ExternalInput")
    whh = nc.dram_tensor("whh", [128, 3 * H], BF16, kind="ExternalInput")
    whead = nc.dram_tensor("whead", [128, A], BF16, kind="ExternalInput")
    brz = nc.dram_tensor("brz", [128, 2], F32, kind="ExternalInput")
    bn_in = nc.dram_tensor("bn_in", [128, 1], F32, kind="ExternalInput")
    bh_col = nc.dram_tensor("bh_col", [A, 1], F32, kind="ExternalInput")
    ident = nc.dram_tensor("ident", [128, 128], BF16, kind="ExternalInput")
    bn_bc = nc.dram_tensor("bn_bc", [128, BL], BF16, kind="ExternalInput")
    h0T = nc.dram_tensor("h0T", [128, BL], BF16, kind="ExternalInput")

    logitsT = nc.dram_tensor("logitsT", [A, S * BL], F32, kind="ExternalOutput")
    hnT = nc.dram_tensor("hnT", [128, BL], F32, kind="ExternalOutput")

    with tile.TileContext(nc) as tc:
        with (
            tc.tile_pool(name="const", bufs=1) as constp,
            tc.tile_pool(name="big", bufs=1) as bigp,
            tc.tile_pool(name="obs", bufs=4) as obsp,
            tc.tile_pool(name="step", bufs=6) as stepp,
            tc.tile_pool(name="lout", bufs=3) as loutp,
            tc.tile_pool(name="pa_ps", bufs=2, space="PSUM") as pa_psp,
            tc.tile_pool(name="n_ps", bufs=1, space="PSUM") as n_psp,
            tc.tile_pool(name="hd_ps", bufs=1, space="PSUM") as hd_psp,
        ):
            # ---- persistent SBUF ----
            wih_sb = constp.tile([128, 2, 3 * H], BF16, tag="wih")
            whh_sb = constp.tile([128, 3 * H], BF16, tag="whh")
            whead_sb = constp.tile([128, A], BF16, tag="whead")
            brz_sb = constp.tile([128, 2], F32, tag="brz")
            bn_in_sb = constp.tile([128, 1], F32, tag="bn_in")
            bh_sb = constp.tile([A, 1], F32, tag="bh")
            ident_sb = constp.tile([128, 128], BF16, tag="ident")
            bnbc_sb = constp.tile([128, BL], BF16, tag="bnbc")

            hT = bigp.tile([128, S + 1, BL], BF16, tag="hT")

            for k in range(2):
                nc.gpsimd.dma_start(wih_sb[:, k, :], wih[k])
            nc.gpsimd.dma_start(whh_sb[:], whh[:])
            nc.gpsimd.dma_start(whead_sb[:], whead[:])
            nc.gpsimd.dma_start(brz_sb[:], brz[:])
            nc.gpsimd.dma_start(bn_in_sb[:], bn_in[:])
            nc.gpsimd.dma_start(bh_sb[:], bh_col[:])
            nc.gpsimd.dma_start(ident_sb[:], ident[:])
            nc.gpsimd.dma_start(bnbc_sb[:], bn_bc[:])
            nc.gpsimd.dma_start(hT[:, 0, :], h0T[:])

            # x-projection PSUM banks per chunk: gate matmuls of the
            # recurrence accumulate directly into these (has_written was
            # set by the phase-A matmuls), so sigmoid reads PSUM and no
            # evacuation pass exists at all.
            pa_tiles = {}
            pa_obs = {}

            def phase_a_dma(c):
                obs_sb = [
                    obsp.tile([128, FREE], BF16, tag=f"obs{k}", name=f"obs{k}")
                    for k in range(2)
                ]
                for k in range(2):
                    nc.gpsimd.dma_start(
                        obs_sb[k][:], obsT[k, :, c * FREE:(c + 1) * FREE]
                    )
                tiles = []
                for gn in ["r", "z", "n"]:
                    ps = pa_psp.tile([128, FREE], F32, tag=f"pa_{gn}",
                                     name=f"pa_{gn}")
                    tiles.append(ps)
                pa_tiles[c] = tiles
                pa_obs[c] = obs_sb

            def phase_a_piece(c, i):
                g, j = divmod(i, 2)
                ps = pa_tiles[c][g]
                obs_sb = pa_obs[c]
                nc.tensor.matmul(
                    ps[:], wih_sb[:, j, g * H:(g + 1) * H],
                    obs_sb[j][:], start=(j == 0), stop=False,
                )

            def step(t):
                c, o = divmod(t, CH)
                pa_r, pa_z, pa_n = pa_tiles[c]
                xr = pa_r[:, o * BL:(o + 1) * BL]
                xz = pa_z[:, o * BL:(o + 1) * BL]
                xn = pa_n[:, o * BL:(o + 1) * BL]
                n_ps = n_psp.tile([128, BL], F32, tag="n")
                r_sb = stepp.tile([128, BL], BF16, tag="r")
                z_sb = stepp.tile([128, BL], BF16, tag="z")
                zc_sb = stepp.tile([128, BL], BF16, tag="zc")
                u_sb = stepp.tile([128, BL], BF16, tag="u")
                q_sb = stepp.tile([128, BL], BF16, tag="q")
                nn_sb = stepp.tile([128, BL], BF16, tag="nn")
                zh_sb = stepp.tile([128, BL], BF16, tag="zh")
                t1_sb = stepp.tile([128, BL], BF16, tag="t1")

                h_prev = hT[:, t, :]
                nc.tensor.matmul(xr, whh_sb[:, 0:H], h_prev,
                                 start=False, stop=False,
                                 skip_group_check=True)
                nc.tensor.matmul(n_ps[:], ident_sb[:], bnbc_sb[:],
                                 start=True, stop=False)
                nc.tensor.matmul(n_ps[:], whh_sb[:, 2 * H:3 * H], h_prev,
                                 start=False, stop=True)
                nc.tensor.matmul(xz, whh_sb[:, H:2 * H], h_prev,
                                 start=False, stop=(o == CH - 1),
                                 skip_group_check=True)

                nc.scalar.activation(r_sb[:], xr, AF.Sigmoid,
                                     bias=brz_sb[:, 0:1])
                # u = (Wn h + b_hn) * r   (b_hn pre-loaded into psum)
                nc.vector.tensor_mul(u_sb[:], n_ps[:], r_sb[:])
                nc.vector.tensor_add(q_sb[:], u_sb[:], xn)
                nc.scalar.activation(nn_sb[:], q_sb[:], AF.Tanh,
                                     bias=bn_in_sb[:, 0:1])

                nc.scalar.activation(z_sb[:], xz, AF.Sigmoid,
                                     bias=brz_sb[:, 1:2])
                nc.vector.tensor_scalar(zc_sb[:], z_sb[:],
                                        -1.0, 1.0, op0=ALU.mult, op1=ALU.add)
                nc.vector.tensor_mul(zh_sb[:], z_sb[:], h_prev)
                nc.vector.tensor_mul(t1_sb[:], zc_sb[:], nn_sb[:])
                nc.vector.tensor_add(hT[:, t + 1, :], t1_sb[:], zh_sb[:])

            head_tiles = {}

            def head_mm(c):
                hd_ps = hd_psp.tile([A, FREE], F32, tag="hd")
                lo_sb = loutp.tile([A, FREE], F32, tag="lo")
                hv = hT[:, 1 + c * CH:1 + (c + 1) * CH, :].rearrange(
                    "p t b -> p (t b)")
                nc.tensor.matmul(hd_ps[:], whead_sb[:], hv, start=True,
                                 stop=True)
                head_tiles[c] = (hd_ps, lo_sb)

            def head_evac(c, j):
                hd_ps, lo_sb = head_tiles[c]
                nc.vector.tensor_scalar_add(
                    lo_sb[:, j * 128:(j + 1) * 128],
                    hd_ps[:, j * 128:(j + 1) * 128], bh_sb[:, 0:1])

            def head_dma(c):
                _, lo_sb = head_tiles[c]
                nc.gpsimd.dma_start(
                    logitsT[:, c * FREE:(c + 1) * FREE], lo_sb[:])

            phase_a_dma(0)
            phase_a_dma(1)
            for i in range(6):
                phase_a_piece(0, i)
            for i in range(6):
                phase_a_piece(1, i)
            for c in range(NCH):
                if c + 2 <= NCH - 1:
                    phase_a_dma(c + 2)
                for o in range(CH):
                    step(c * CH + o)
                    if c + 2 <= NCH - 1 and o < 6:
                        phase_a_piece(c + 2, o)
                    if c >= 1:
                        if o == 10:
                            head_mm(c - 1)
                        elif 11 <= o <= 14:
                            head_evac(c - 1, o - 11)
                        elif o == 15:
                            head_dma(c - 1)
            head_mm(NCH - 1)
            for j in range(4):
                head_evac(NCH - 1, j)
            head_dma(NCH - 1)
            hn_sb = constp.tile([128, BL], F32, tag="hn")
            nc.vector.tensor_copy(hn_sb[:], hT[:, S, :])
            nc.gpsimd.dma_start(hnT[:], hn_sb[:])

    _split_multi_waits(nc)
    return nc


_WAIT_TEMPLATE = None


def _get_wait_template():
    global _WAIT_TEMPLATE
    if _WAIT_TEMPLATE is None:
        scratch = bass.Bass()
        with scratch.semaphore() as sem:
            bi = scratch.vector.wait_ge(sem, 1)
        _WAIT_TEMPLATE = bi.ins
    return _WAIT_TEMPLATE


def _split_multi_waits(nc):
    """Walrus on this toolchain rejects instructions with >1 sync wait.
    Move extra waits onto standalone same-engine EventSemaphore
    instructions directly before the instruction (engines are in-order,
    so semantics are preserved).  DMAs must be SWDGE (gpsimd-issued) for
    this to be sound; HWDGE descriptors are not gated by engine order."""
    import copy

    tmpl = _get_wait_template()
    f = nc.m.functions[0]
    n = [0]
    for blk in f.blocks:
        out = []
        for inst in blk.instructions:
            si = inst.sync_info
            waits = list(si.on_wait or []) if si else []
            if len(waits) > 1:
                if type(inst).__name__ == "InstDMACopy":
                    assert str(inst.engine) in ("EngineType.Pool",), (
                        f"multi-wait HWDGE DMA {inst.name} engine {inst.engine}"
                    )
                for w in waits[:-1]:
                    wi = copy.deepcopy(tmpl)
                    n[0] += 1
                    wi.name = f"I-wsplit-{n[0]}"
                    wi.engine = inst.engine
                    wi.sync_info = mybir.SyncInfo(on_wait=[w], on_update=[])
                    out.append(wi)
                inst.sync_info = mybir.SyncInfo(
                    on_wait=[waits[-1]], on_update=list(si.on_update or [])
                )
            out.append(inst)
        try:
            blk.instructions[:] = out
        except TypeError:
            blk.instructions = out
    return n[0]


def _prep_shared(W_ih, W_hh, b_ih, b_hh, W_head, b_head):
    wih = np.ascontiguousarray(W_ih.T.reshape(2, 128, 3 * H)).astype(nbf)
    whh = np.ascontiguousarray(W_hh.T).astype(nbf)
    whead = np.ascontiguousarray(W_head.T).astype(nbf)
    brz = np.stack(
        [b_ih[0:H] + b_hh[0:H], b_ih[H:2 * H] + b_hh[H:2 * H]], axis=1
    ).astype(np.float32)
    bn_in = np.ascontiguousarray(
        b_ih[2 * H:3 * H].reshape(H, 1)).astype(np.float32)
    bh = np.ascontiguousarray(b_head.reshape(A, 1)).astype(np.float32)
    ident = np.eye(128, dtype=nbf)
    bn_bc = np.ascontiguousarray(
        np.broadcast_to(b_hh[2 * H:3 * H].reshape(H, 1), (H, BL))).astype(nbf)
    return dict(wih=wih, whh=whh, whead=whead,
                brz=brz, bn_in=bn_in, bh_col=bh, ident=ident, bn_bc=bn_bc)


last_in_maps = None
_nc_cache = []


def _nc_cache_get():
    if not _nc_cache:
        _nc_cache.append(build_nc())
    return _nc_cache[0]


def kernel(obs_seq, h0, W_ih, W_hh, b_ih, b_hh, W_head, b_head):
    global last_in_maps
    obs_seq = np.asarray(obs_seq, dtype=np.float32)
    h0 = np.asarray(h0, dtype=np.float32)
    shared = _prep_shared(
        np.asarray(W_ih, np.float32), np.asarray(W_hh, np.float32),
        np.asarray(b_ih, np.float32), np.asarray(b_hh, np.float32),
        np.asarray(W_head, np.float32), np.asarray(b_head, np.float32))

    in_maps = []
    for k in range(NCORES):
        ob = obs_seq[k * BL:(k + 1) * BL]          # (32, 512, 256)
        obT = np.ascontiguousarray(ob.transpose(2, 1, 0))  # (256, 512, 32)
        obT = obT.reshape(2, 128, S * BL).astype(nbf)
        h0T = np.ascontiguousarray(h0[0, k * BL:(k + 1) * BL, :].T).astype(nbf)
        m = dict(shared)
        m["obsT"] = obT
        m["h0T"] = h0T
        in_maps.append(m)
    last_in_maps = in_maps

    nc = _nc_cache_get()

    res = run_bass_kernel_spmd(nc, in_maps, core_ids=list(range(NCORES)))
    logits = np.empty((B, S, A), dtype=np.float32)
    hn = np.empty((1, B, H), dtype=np.float32)
    for k in range(NCORES):
        lt = res.results[k]["logitsT"].reshape(A, S, BL)
        logits[k * BL:(k + 1) * BL] = lt.transpose(2, 1, 0)
        hn[0, k * BL:(k + 1) * BL, :] = res.results[k]["hnT"].T
    return logits, hn
